# revision 22
# baseline (speedup 1.0000x reference)
"""BTT (block tensor-train) forward kernel for 8 TRN2 NeuronCores.

Problem: out[b, y*64+i] = sum_{j,x,r} x[b, j*64+x] * c0[j,x,i,r] * c1[j,y,i,r]
with B=8192, D=64, R=4 (nn_BTT_56788057588464).

Sharding: data-parallel over batch (1024 rows/core), TT cores replicated.

Per-core device pipeline, per 128-row batch tile (all bf16 compute,
fp32 PSUM accumulate):
  s1   : 64 matmuls (xT stationary, K=x=64) -> PSUM z_j[b, (i,r)],
         four j's per 2-bank PSUM tile, one batched evac copy each
  evac1: DVE/ACT batched copy+cast -> z_sb[b, j*256+i*4+r] (bf16, j-major,
         contiguous dst)
  trans: 128 PE transpose-mode matmuls, strided src [b, (j-half,r)@i]
         -> PSUM [(j,r)-half, b] per (i,h); 8 chunks chained per bank tile
  evac2: batched copy -> z2[(j32,r), i*256 + h*128 + b] (bf16)
  s2   : 128 matmuls (c1 stationary, K=(j,r) halves accumulated, M=y=64,
         col-packed i-parity pairs) -> PSUM out[(i%2)*64+y, b]
  dma  : PSUM -> HBM directly (no SBUF bounce for the output)

Host does: x transpose/bf16-cast/layout, weight rearrangement, output
unpermute, bias add.
"""

import numpy as np

import concourse.bass as bass
import concourse.mybir as mybir
from concourse.bass_utils import run_bass_kernel_spmd
from concourse.tile import TileContext

N_CORES = 8
# test-harness hooks (harness calls kernel() with defaults; test.py flips TRACE
# to profile the NEFF and reads LAST_RESULT.exec_time_ns)
TRACE = False
LAST_RESULT = None
B = 8192
D = 64
R = 4
BC = B // N_CORES          # batch per core = 1024
BT = 128                   # batch tile (SBUF partitions)
NBT = BC // BT             # batch tiles per core = 8
IR = D * R                 # 256 = (i, r) per j
JR2 = D * R // 2           # 128 = (j, r) half

BF16 = mybir.dt.bfloat16
F32 = mybir.dt.float32


def _split_multi_waits(nc: bass.Bass):
    """This container's walrus accepts only ONE sync-wait per instruction.
    Tile routinely emits several (e.g. a matmul waiting on two DMA lanes).
    Move the extra waits onto nofuse nops inserted just before, on the same
    engine queue -- sequential waits on one queue are equivalent."""
    fn = nc.m.functions[0]
    for bb in fn.blocks:
        new_insts = []
        changed = False
        for ins in bb.instructions:
            si = ins.sync_info
            if si is not None and si.on_wait and len(si.on_wait) > 1:
                changed = True
                waits = list(si.on_wait)
                del si.on_wait[:]
                si.on_wait.append(waits[-1])
                for k, w in enumerate(waits[:-1]):
                    nop = mybir.InstNoOp(
                        name=f"{ins.name}-wsplit{k}",
                        sync_info=mybir.SyncInfo(on_wait=[w], on_update=[]),
                        bass_nofuse=True,
                        engine=ins.engine,
                    )
                    nc.register_instruction(nop)
                    new_insts.append(nop)
            new_insts.append(ins)
        if changed:
            bb.instructions = new_insts


def build_nc() -> bass.Bass:
    nc = bass.Bass()

    xt_d = nc.declare_dram_parameter("xt", [NBT, 128, 32 * BT], BF16, isOutput=False)
    c0_d = nc.declare_dram_parameter("c0", [128, 32 * IR], BF16, isOutput=False)
    c1_d = nc.declare_dram_parameter("c1", [2, 128, D * D], BF16, isOutput=False)
    id_d = nc.declare_dram_parameter("ident", [128, 128], BF16, isOutput=False)
    out_d = nc.declare_dram_parameter("out", [NBT, BT, D * D], F32, isOutput=True)

    with TileContext(nc) as tc:
        with (
            tc.tile_pool(name="const", bufs=1) as cpool,
            tc.tile_pool(name="xt", bufs=2) as xpool,
            tc.tile_pool(name="z", bufs=2) as zpool,
            tc.tile_pool(name="z2", bufs=2) as z2pool,
            tc.tile_pool(name="psz", bufs=2, space="PSUM") as pszpool,
            tc.tile_pool(name="pstr", bufs=2, space="PSUM") as pstrpool,
            tc.tile_pool(name="psout", bufs=2, space="PSUM") as psopool,
        ):
            c0_sb = cpool.tile([128, 32 * IR], BF16, tag="c0")
            for q in range(4):
                nc.sync.dma_start(
                    c0_sb[:, q * 2048:(q + 1) * 2048],
                    c0_d[:, q * 2048:(q + 1) * 2048],
                )
            c1_sb = [
                cpool.tile([128, D * D], BF16, tag=f"c1_{h}", name=f"c1_{h}")
                for h in (0, 1)
            ]
            for h in (0, 1):
                nc.sync.dma_start(c1_sb[h][:], c1_d[h])
            ident = cpool.tile([128, 128], BF16, tag="ident")
            nc.sync.dma_start(ident[:], id_d[:])

            cnt = {"f32": 0, "bf16": 0}

            def evac(dst_ap, src_ap, kind):
                # DVE reads bf16 PSUM at ~0.78 ns/elem vs ~1.3 for fp32;
                # ACT is ~1.2 for both. Give DVE all bf16 copies plus a
                # small share of fp32; ACT takes the rest.
                if kind == "bf16":
                    nc.vector.tensor_copy(dst_ap, src_ap)
                    return
                cnt["f32"] += 1
                if cnt["f32"] % 8 in (0, 3, 6):
                    nc.vector.tensor_copy(dst_ap, src_ap)
                else:
                    nc.scalar.copy(dst_ap, src_ap)

            def s1_emitter(bt):
                """s1: 16 same-parity 4-j chains -> batched evacs into z."""
                xt = xpool.tile([128, 32 * BT], BF16, tag="xt", name="xt")
                nc.sync.dma_start(xt[:], xt_d[bt])
                # z[b, i*256 + j*4 + r], i-major (contiguous transpose reads)
                z = zpool.tile([BT, D * D * R], BF16, tag="z", name="z")
                z_of[bt] = z
                zq = z[:].rearrange(
                    "p (i jq jl par r) -> p par jq i jl r",
                    i=D, jq=8, jl=4, par=2, r=R,
                )
                for par in (0, 1):
                    for jq in range(8):
                        pz = pszpool.tile([BT, 4 * IR], F32, tag="pz", name="pz")
                        for jl in range(4):
                            jp = 4 * jq + jl
                            nc.tensor.matmul(
                                pz[:, jl * IR:(jl + 1) * IR],
                                xt[par * 64:(par + 1) * 64,
                                   jp * BT:(jp + 1) * BT],
                                c0_sb[par * 64:(par + 1) * 64,
                                      jp * IR:(jp + 1) * IR],
                                start=(jl % 2 == 0),
                                stop=(jl % 2 == 1),
                            )
                        evac(
                            zq[:, par, jq],
                            pz[:].rearrange(
                                "p (jl i r) -> p i jl r", jl=4, i=D, r=R
                            ),
                            "f32",
                        )
                        yield
                return

            def tr_s2_emitter(bt, z, out_sb):
                """transposes + s2 for one tile's z (emitted interleaved)."""
                for ih in (0, 1):             # i halves, 32 i's each
                    # z2[(j32,r), i_loc*256 + h*128 + b]
                    z2 = z2pool.tile([JR2, 32 * IR], BF16, tag="z2", name="z2")
                    for ig in range(8):       # groups of 4 i within the half
                        pt = pstrpool.tile([JR2, 8 * BT], BF16, tag="pt",
                                           name="pt")
                        for ii in range(4):
                            i = ih * 32 + ig * 4 + ii
                            for h in (0, 1):
                                ci = 2 * ii + h
                                nc.tensor.matmul(
                                    pt[:, ci * BT:(ci + 1) * BT],
                                    z[:, i * IR + h * JR2:
                                      i * IR + (h + 1) * JR2],
                                    ident[:],
                                    is_transpose=True,
                                    start=(ci == 0),
                                    stop=(ci == 7),
                                )
                        evac(z2[:, ig * 4 * IR:(ig + 1) * 4 * IR], pt[:],
                             "bf16")
                        yield
                    for og in range(4):       # 4 ipairs (512 out cols) each
                        po = psopool.tile([BT, 4 * BT], F32, tag="po",
                                          name="po")
                        for ip4 in range(4):
                            ipair = og * 4 + ip4
                            for h in (0, 1):
                                for par in (0, 1):
                                    il = 2 * ipair + par
                                    i = ih * 32 + il
                                    # start=True zeros the whole 2KB bank:
                                    # only the first matmul per partition
                                    # range starts.
                                    nc.tensor.matmul(
                                        po[par * 64:(par + 1) * 64,
                                           ip4 * BT:(ip4 + 1) * BT],
                                        c1_sb[h][:, i * D:(i + 1) * D],
                                        z2[:, il * IR + h * BT:
                                           il * IR + (h + 1) * BT],
                                        start=(ip4 == 0 and h == 0),
                                        stop=(ip4 == 3 and h == 1),
                                        # par=1 writes partitions 64-127; the
                                        # sim's region tracker ignores the
                                        # partition base and flags a false
                                        # group conflict with par=0.
                                        skip_group_check=(par == 1),
                                    )
                        evac(
                            out_sb[:, (ih * 4 + og) * 4 * BT:
                                   (ih * 4 + og + 1) * 4 * BT],
                            po[:],
                            "f32",
                        )
                        nc.sync.dma_start(
                            out_d[bt][:, (ih * 4 + og) * 4 * BT:
                                      (ih * 4 + og + 1) * 4 * BT],
                            out_sb[:, (ih * 4 + og) * 4 * BT:
                                   (ih * 4 + og + 1) * 4 * BT],
                        )
                        yield
                return

            # Software-pipelined emission: interleave s1(bt) with
            # transposes+s2(bt-1) so the PE stream always contains normal
            # matmuls (keeps the HAM clock at 2.4 GHz through the
            # transpose-heavy phases) and both stay dense.
            z_of = {}
            prev = None
            for bt in range(NBT + 1):
                cur = None
                if bt < NBT:
                    cur = s1_emitter(bt)
                    # peek: the emitter allocates xt/z on first next();
                    # grab z by running the generator lazily via a wrapper
                gens = []
                if cur is not None:
                    gens.append(("s1", cur))
                if prev is not None:
                    gens.append(("ts", prev))
                # round-robin: 2 ts groups per 1 s1 group (24 vs 16 total)
                alive = {k: g for k, g in gens}
                while alive:
                    for key, ratio in (("ts", 2), ("s1", 1)):  # 2 ts : 1 s1
                        g = alive.get(key)
                        if g is None:
                            continue
                        for _ in range(ratio):
                            try:
                                next(g)
                            except StopIteration:
                                del alive[key]
                                break
                if bt < NBT:
                    out_sb = xpool.tile([BT, D * D], F32, tag="osb",
                                        name="osb")
                    prev = tr_s2_emitter(bt, z_of.pop(bt), out_sb)
                else:
                    prev = None

    _split_multi_waits(nc)
    return nc


def _prep_core_inputs(xc, c0_arr, c1_arr, ident):
    """Host-side layout transforms for one core's batch shard xc (BC, 4096)."""
    import ml_dtypes

    bf = ml_dtypes.bfloat16
    # xt[bt, (par,x), jp*128+b] = xc[bt*128+b, (2*jp+par)*64 + x]
    xt = (
        xc.reshape(NBT, BT, 32, 2, 64)      # bt, b, jp, par, x
        .transpose(0, 3, 4, 2, 1)            # bt, par, x, jp, b
        .reshape(NBT, 128, 32 * BT)
        .astype(bf)
    )
    return {"xt": xt, "c0": c0_arr, "c1": c1_arr, "ident": ident}


def kernel(x, core0, core1, bias):
    import ml_dtypes

    bf = ml_dtypes.bfloat16
    x = np.asarray(x, np.float32)
    c0 = np.asarray(core0, np.float32).reshape(D, D, D, R)   # j, x, i, r
    c1 = np.asarray(core1, np.float32).reshape(D, D, D, R)   # j, y, i, r
    bias = np.asarray(bias, np.float32)

    # c0_arr[(par,x), jp*256 + i*4 + r] = c0[2*jp+par, x, i, r]
    c0_arr = (
        c0.reshape(32, 2, D, D, R).transpose(1, 2, 0, 3, 4).reshape(128, 32 * IR)
        .astype(bf)
    )
    # c1_arr[h, jl*4+r, i*64+y] = c1[h*32+jl, y, i, r]
    c1_arr = (
        c1.reshape(2, 32, D, D, R).transpose(0, 1, 4, 3, 2).reshape(2, 128, D * D)
        .astype(bf)
    )
    ident = np.eye(128, dtype=np.float32).astype(bf)

    nc = build_nc()
    in_maps = [
        _prep_core_inputs(x[c * BC:(c + 1) * BC], c0_arr, c1_arr, ident)
        for c in range(N_CORES)
    ]
    res = run_bass_kernel_spmd(
        nc, in_maps, core_ids=list(range(N_CORES)), trace=TRACE
    )
    global LAST_RESULT
    LAST_RESULT = res

    # out_dev[bt, par*64+y, ipair*128+b] = out[bt*128+b, y*64 + 2*ipair+par]
    outs = []
    for c in range(N_CORES):
        od = res.results[c]["out"].reshape(NBT, 2, D, 32, BT)  # bt, par, y, ipair, b
        oc = od.transpose(0, 4, 2, 3, 1).reshape(BC, D * D)    # bt,b , y, ipair, par
        outs.append(oc)
    out = np.concatenate(outs, axis=0)
    return (out + bias[None, :]).astype(np.float32)
